# Initial kernel scaffold
#
"""Bass/Tile kernel for nn_Attention_9234179687166 on 8 TRN2 NeuronCores.

Reference computation per batch b (B=32, L=K=D=1024):
    q      = query @ W_in.T                    # [L, D]
    scores = q @ context.T                     # [L, K]
    w      = masked_softmax(scores, mask)      # multiplicative mask + renorm
    mix    = w @ context                       # [L, D]
    out    = tanh(concat([mix, q]) @ W_out.T)  # [L, D]

Sharding: data-parallel over batch, 4 batches per core, weights replicated.

Per-core program layout (contraction dim always on partitions):
    W_inT[d,e], W_outT[c,d] built once by PE transpose (W_out as bf16).
    Per batch: ctxT[e,k] (fp32, PE transpose), ctx_bf[k,d'] (bf16 cast).
    Per l-half: qT[d,l] (PE transpose), step1 -> qTr[e,l] (fp32r matmuls),
    step2 scores in PSUM (fp32r), masked softmax (DVE+ACT, see below),
    w transposed to wT[k,l] (bf16), step4 -> mixT[d',l] (bf16), step5
    out[l,d] (bf16) + tanh, DMA out.

Masked softmax (mask m in {0,1}, scores s):
    reference: w0 = softmax(s*m); w = w0*m / (sum(w0*m) + 1e-13)
    Softmax is shift invariant, so with u = (s + 4096)*m  (masked -> 0),
    e = exp(u - max(u)) has masked lanes exp(-~4096) == 0 exactly, and
    w = e / sum(e) matches the reference up to the +1e-13*Z/S ~ 1e-10 term.
"""

import sys

sys.path.insert(0, "/opt/trn_rl_repo")

import numpy as np

P = 128
D = 1024
TWO_D = 2048
DT = D // P      # 8 tiles over D
CT = TWO_D // P  # 16 tiles over 2D
LARGE = 4096.0
N_CORES = 8
B_FULL = 32
NB = B_FULL // N_CORES  # batches per core

_prog_cache = {}


def build_program(nb, L, K=1024):
    import concourse.mybir as mybir
    import concourse.tile as tile
    from concourse import bacc
    from concourse.masks import make_identity

    f32 = mybir.dt.float32
    f32r = mybir.dt.float32r
    bf16 = mybir.dt.bfloat16
    i32 = mybir.dt.int32
    Alu = mybir.AluOpType
    Act = mybir.ActivationFunctionType
    KT = K // P
    LH = min(512, L)      # l-half width (free dim of step1/4 matmuls)
    NHALF = L // LH
    LJ = LH // P          # 128-row l tiles per half
    KH = K // 512         # 512-wide k chunks for the scores matmul

    nc = bacc.Bacc("TRN2", target_bir_lowering=False, debug=False,
                   num_devices=N_CORES)
    q_d = nc.dram_tensor("query", [nb, L, D], f32, kind="ExternalInput")
    c_d = nc.dram_tensor("context", [nb, K, D], f32, kind="ExternalInput")
    m_d = nc.dram_tensor("mask", [nb, L, K], i32, kind="ExternalInput")
    win_d = nc.dram_tensor("W_in", [D, D], f32, kind="ExternalInput")
    wout_d = nc.dram_tensor("W_out", [D, TWO_D], f32, kind="ExternalInput")
    out_d = nc.dram_tensor("out", [nb, L, D], f32, kind="ExternalOutput")

    with tile.TileContext(nc) as tc:
        with (
            tc.tile_pool(name="const", bufs=1) as constp,
            tc.tile_pool(name="wres", bufs=1) as wres,
            tc.tile_pool(name="ps_big", bufs=2, space="PSUM") as ps_big,
            tc.tile_pool(name="ps_mm", bufs=2, space="PSUM") as ps_mm,
            tc.tile_pool(name="ps_out", bufs=2, space="PSUM") as ps_out,
        ):
            ident = constp.tile([P, P], f32)
            make_identity(nc, ident)
            ident_bf = constp.tile([P, P], bf16)
            nc.vector.tensor_copy(ident_bf[:], ident[:])

            W_inT = wres.tile([P, DT, D], f32)       # [d_in, d_out, e]
            W_outT = wres.tile([P, CT, D], bf16)     # [c_in, c_out, d]

            with tc.tile_pool(name="setup", bufs=2) as sp:
                for ei in range(DT):
                    nat = sp.tile([P, D], f32, tag="snat")
                    nc.sync.dma_start(nat[:], win_d[ei * P:(ei + 1) * P, :])
                    for di in range(DT):
                        tp = ps_mm.tile([P, P], f32, tag="mm")
                        nc.tensor.transpose(tp[:], nat[:, di * P:(di + 1) * P], ident[:])
                        nc.vector.tensor_copy(W_inT[:, di, ei * P:(ei + 1) * P], tp[:])
                for di in range(DT):
                    nat2 = sp.tile([P, TWO_D], f32, tag="snat2")
                    nc.sync.dma_start(nat2[:], wout_d[di * P:(di + 1) * P, :])
                    natb = sp.tile([P, TWO_D], bf16, tag="snatb")
                    nc.scalar.activation(natb[:], nat2[:], Act.Copy)
                    for ci in range(CT):
                        tpb = ps_mm.tile([P, P], bf16, tag="mm")
                        nc.tensor.transpose(tpb[:], natb[:, ci * P:(ci + 1) * P], ident_bf[:])
                        nc.vector.tensor_copy(W_outT[:, ci, di * P:(di + 1) * P], tpb[:])

            with (
                tc.tile_pool(name="ctx", bufs=1) as ctxp,
                tc.tile_pool(name="acts", bufs=1) as actsp,
                tc.tile_pool(name="rot", bufs=3) as natp,
                tc.tile_pool(name="sm", bufs=2) as smp,
            ):
                for b in range(nb):
                    # ---- context: transpose to ctxT (f32) + cast to bf16 ----
                    ctxT = ctxp.tile([P, DT, K], f32, tag="ctxT")     # [e_in, e_out, k]
                    ctx_bf = ctxp.tile([P, KT, D], bf16, tag="ctxbf")  # [k_in, k_out, d']
                    for ki in range(KT):
                        nat = natp.tile([P, D], f32, tag="nat")
                        nc.sync.dma_start(nat[:], c_d[b, ki * P:(ki + 1) * P, :])
                        nc.scalar.activation(ctx_bf[:, ki, :], nat[:], Act.Copy)
                        for ei in range(DT):
                            tp = ps_mm.tile([P, P], f32, tag="mm")
                            nc.tensor.transpose(tp[:], nat[:, ei * P:(ei + 1) * P], ident[:])
                            nc.vector.tensor_copy(ctxT[:, ei, ki * P:(ki + 1) * P], tp[:])

                    for h in range(NHALF):
                        l0 = h * LH
                        # ---- query transpose: qT[d, l] ----
                        qT = actsp.tile([P, DT, LH], f32, tag="qT")
                        for lj in range(LJ):
                            nat = natp.tile([P, D], f32, tag="nat")
                            nc.sync.dma_start(nat[:], q_d[b, l0 + lj * P: l0 + (lj + 1) * P, :])
                            for di in range(DT):
                                tp = ps_mm.tile([P, P], f32, tag="mm")
                                nc.tensor.transpose(tp[:], nat[:, di * P:(di + 1) * P], ident[:])
                                nc.vector.tensor_copy(qT[:, di, lj * P:(lj + 1) * P], tp[:])

                        # ---- step 1: qTr[e, l] = W_inT.T @ qT (fp32r) ----
                        qTr = actsp.tile([P, DT, LH], f32, tag="qTr")
                        qTr_bf = actsp.tile([P, DT, LH], bf16, tag="qTrbf")
                        for ei in range(DT):
                            psq = ps_mm.tile([P, LH], f32, tag="mm")
                            for di in range(DT):
                                nc.tensor.matmul(
                                    psq[:],
                                    W_inT[:, di, ei * P:(ei + 1) * P].bitcast(f32r),
                                    qT[:, di, :].bitcast(f32r),
                                    start=(di == 0), stop=(di == DT - 1),
                                )
                            nc.vector.tensor_copy(qTr[:, ei, :], psq[:])
                            nc.scalar.activation(qTr_bf[:, ei, :], psq[:], Act.Copy)

                        # ---- step 2 + masked softmax + transpose w ----
                        wT = actsp.tile([P, KT, LH], bf16, tag="wT")
                        for lj in range(LJ):
                            pss = ps_big.tile([P, K], f32, tag="scores")
                            for ei in range(DT):
                                for kh in range(KH):
                                    nc.tensor.matmul(
                                        pss[:, kh * 512:(kh + 1) * 512],
                                        qTr[:, ei, lj * P:(lj + 1) * P].bitcast(f32r),
                                        ctxT[:, ei, kh * 512:(kh + 1) * 512].bitcast(f32r),
                                        start=(ei == 0), stop=(ei == DT - 1),
                                    )
                            mi = smp.tile([P, K], i32, tag="mask")
                            nc.sync.dma_start(mi[:], m_d[b, l0 + lj * P: l0 + (lj + 1) * P, :])
                            st = smp.tile([P, 4], f32, tag="stats")
                            # u = (s + LARGE) * m, in place in PSUM
                            nc.vector.scalar_tensor_tensor(
                                pss[:], pss[:], LARGE, mi[:], op0=Alu.add, op1=Alu.mult)
                            nc.vector.tensor_reduce(
                                st[:, 0:1], pss[:], axis=mybir.AxisListType.X,
                                op=Alu.max, negate=True)
                            e_sb = smp.tile([P, K], f32, tag="e")
                            nc.scalar.activation(
                                e_sb[:], pss[:], Act.Exp,
                                bias=st[:, 0:1], accum_out=st[:, 1:2])
                            nc.vector.reciprocal(st[:, 2:3], st[:, 1:2])
                            w_bf = smp.tile([P, K], bf16, tag="w")
                            nc.vector.tensor_scalar_mul(w_bf[:], e_sb[:], st[:, 2:3])
                            for ki in range(KT):
                                tpb = ps_mm.tile([P, P], bf16, tag="mm")
                                nc.tensor.transpose(tpb[:], w_bf[:, ki * P:(ki + 1) * P], ident_bf[:])
                                nc.vector.tensor_copy(wT[:, ki, lj * P:(lj + 1) * P], tpb[:])

                        # ---- step 4: mixT[d', l] = ctx_bf.T @ wT (bf16) ----
                        mixT = actsp.tile([P, DT, LH], bf16, tag="mixT")
                        for di in range(DT):
                            psm = ps_mm.tile([P, LH], f32, tag="mm")
                            for ki in range(KT):
                                nc.tensor.matmul(
                                    psm[:],
                                    ctx_bf[:, ki, di * P:(di + 1) * P],
                                    wT[:, ki, :],
                                    start=(ki == 0), stop=(ki == KT - 1),
                                )
                            nc.scalar.activation(mixT[:, di, :], psm[:], Act.Copy)

                        # ---- step 5: out[l, d] = tanh(combinedT.T @ W_outT) ----
                        for lj in range(LJ):
                            for dh in range(D // 512):
                                pso = ps_out.tile([P, 512], f32, tag="out")
                                for ci in range(CT):
                                    if ci < DT:
                                        lhs = mixT[:, ci, lj * P:(lj + 1) * P]
                                    else:
                                        lhs = qTr_bf[:, ci - DT, lj * P:(lj + 1) * P]
                                    nc.tensor.matmul(
                                        pso[:], lhs,
                                        W_outT[:, ci, dh * 512:(dh + 1) * 512],
                                        start=(ci == 0), stop=(ci == CT - 1),
                                    )
                                nc.scalar.activation(pso[:], pso[:], Act.Tanh)
                                nc.sync.dma_start(
                                    out_d[b, l0 + lj * P: l0 + (lj + 1) * P,
                                          dh * 512:(dh + 1) * 512],
                                    pso[:])

    nc.compile()
    return nc


def _get_program(nb, L):
    key = (nb, L)
    if key not in _prog_cache:
        _prog_cache[key] = build_program(nb, L)
    return _prog_cache[key]


def kernel(query, context, mask, W_in, W_out):
    from concourse.bass_utils import run_bass_kernel_spmd

    query = np.ascontiguousarray(query, dtype=np.float32)
    context = np.ascontiguousarray(context, dtype=np.float32)
    W_in = np.ascontiguousarray(W_in, dtype=np.float32)
    W_out = np.ascontiguousarray(W_out, dtype=np.float32)
    B, L, _ = query.shape
    mask3 = np.ascontiguousarray(mask.reshape(B, L, -1), dtype=np.int32)

    nb = B // N_CORES
    nc = _get_program(nb, L)
    in_maps = []
    for c in range(N_CORES):
        b0 = c * nb
        in_maps.append({
            "query": query[b0:b0 + nb],
            "context": context[b0:b0 + nb],
            "mask": mask3[b0:b0 + nb],
            "W_in": W_in,
            "W_out": W_out,
        })
    res = run_bass_kernel_spmd(nc, in_maps, core_ids=list(range(N_CORES)))
    out = np.concatenate([r["out"] for r in res.results], axis=0)
    return out


# revision 15
# speedup vs baseline: 1.8235x; 1.8235x over previous
"""Bass/Tile kernel for nn_Attention_9234179687166 on 8 TRN2 NeuronCores.

Reference computation per batch b (B=32, L=K=D=1024):
    q      = query @ W_in.T                    # [L, D]
    scores = q @ context.T                     # [L, K]
    w      = masked_softmax(scores, mask)      # multiplicative mask + renorm
    mix    = w @ context                       # [L, D]
    out    = tanh(concat([mix, q]) @ W_out.T)  # [L, D]

Sharding: data-parallel over batch, 4 batches per core, weights replicated.

Per-core program layout (contraction dim always on partitions):
    W_inT[d,e], W_outT[c,d] built once by PE transpose (W_out as bf16).
    Per batch: ctxT[e,k] (fp32r, PE transpose), ctx_bf[k,d'] (bf16 cast).
    Per l-half: qT[d,l] (PE transpose), step1 -> qTr[e,l] (fp32r matmuls),
    step2 scores in PSUM (fp32r), masked softmax (DVE+ACT, see below),
    w transposed to wT[k,l] (bf16), step4 -> mixT[d',l] (bf16), step5
    out[l,d] (bf16) + tanh, DMA out.

Transposes are 4-packed: four 128x128 PE transposes land in one [128,512]
PSUM tile and leave with a single grouped copy (alternating DVE/ACT), which
cuts copy count and PSUM slot churn 4x. The w-transposes of each l-tile are
deferred one iteration, and the next half's query transposes (or next
batch's context stage) are emitted inside the last softmax tail, so the PE
never sits idle waiting for the softmax chain.

Masked softmax (mask m in {0,1}, scores s):
    reference: w0 = softmax(s*m); w = w0*m / (sum(w0*m) + 1e-13)
    Softmax is shift invariant, so with u = (s + 4096)*m  (masked -> 0),
    e = exp(u - max(u)) has masked lanes exp(-~4096) == 0 exactly, and
    w = e / sum(e) matches the reference up to the +1e-13*Z/S ~ 1e-10 term.
"""

import sys

sys.path.insert(0, "/opt/trn_rl_repo")

import numpy as np

P = 128
D = 1024
TWO_D = 2048
DT = D // P      # 8 tiles over D
CT = TWO_D // P  # 16 tiles over 2D
LARGE = 4096.0
N_CORES = 8
B_FULL = 32
NB = B_FULL // N_CORES  # batches per core

_prog_cache = {}
last_results = None  # BassKernelResults of the most recent kernel() call


def build_program(nb, L, K=1024):
    import concourse.mybir as mybir
    import concourse.tile as tile
    from concourse import bacc
    from concourse.masks import make_identity

    f32 = mybir.dt.float32
    f32r = mybir.dt.float32r
    bf16 = mybir.dt.bfloat16
    i32 = mybir.dt.int32
    Alu = mybir.AluOpType
    Act = mybir.ActivationFunctionType
    KT = K // P
    LH = min(512, L)      # l-half width (free dim of step1/4 matmuls)
    NHALF = L // LH
    LJ = LH // P          # 128-row l tiles per half
    KH = K // 512         # 512-wide k chunks for the scores matmul

    nc = bacc.Bacc("TRN2", target_bir_lowering=False, debug=False,
                   num_devices=N_CORES)
    q_d = nc.dram_tensor("query", [nb, L, D], f32, kind="ExternalInput")
    c_d = nc.dram_tensor("context", [nb, K, D], f32, kind="ExternalInput")
    m_d = nc.dram_tensor("mask", [nb, L, K], i32, kind="ExternalInput")
    win_d = nc.dram_tensor("W_in", [D, D], f32, kind="ExternalInput")
    wout_d = nc.dram_tensor("W_out", [D, TWO_D], f32, kind="ExternalInput")
    out_d = nc.dram_tensor("out", [nb, L, D], f32, kind="ExternalOutput")

    copy_flip = [0]

    def grouped_copy(nc, dst_ap, src_ap):
        # Alternate psum->sbuf copies between DVE and ACT to halve the
        # per-engine copy latency chain behind the PE transposes.
        if copy_flip[0] % 2 == 0:
            nc.vector.tensor_copy(dst_ap, src_ap)
        else:
            nc.scalar.activation(dst_ap, src_ap, mybir.ActivationFunctionType.Copy)
        copy_flip[0] += 1

    with tile.TileContext(nc) as tc:
        with (
            tc.tile_pool(name="const", bufs=1) as constp,
            tc.tile_pool(name="wres", bufs=1) as wres,
            tc.tile_pool(name="ps_big", bufs=2, space="PSUM") as ps_big,
            tc.tile_pool(name="ps_mm", bufs=2, space="PSUM") as ps_mm,
            tc.tile_pool(name="ps_out", bufs=2, space="PSUM") as ps_out,
        ):
            ident = constp.tile([P, P], f32)
            make_identity(nc, ident)
            ident_bf = constp.tile([P, P], bf16)
            nc.vector.tensor_copy(ident_bf[:], ident[:])

            W_inT = wres.tile([P, DT, D], f32r)       # [d_in, d_out, e]
            W_outT = wres.tile([P, CT, D], bf16)      # [c_in, c_out, d]

            def transpose_pack4(nc, dst_tile, dst_t0, dst_col0, src_ap_fn, n, idn,
                                dtype):
                """n transposes (groups of up to 4) of 128x128 slices.
                src_ap_fn(i) gives the i-th source slice; results land in
                dst_tile[:, dst_t0+i, dst_col0:dst_col0+128]."""
                g = 0
                while g < n:
                    gn = min(4, n - g)
                    tp = ps_mm.tile([P, 4 * P], dtype, tag="mm")
                    for i in range(gn):
                        nc.tensor.transpose(
                            tp[:, i * P:(i + 1) * P], src_ap_fn(g + i), idn[:])
                    grouped_copy(
                        nc,
                        dst_tile[:, dst_t0 + g:dst_t0 + g + gn,
                                 dst_col0:dst_col0 + P],
                        tp[:, :gn * P],
                    )
                    g += gn

            with tc.tile_pool(name="setup", bufs=3) as sp:
                for ei in range(DT):
                    nat = sp.tile([P, D], f32, tag="snat")
                    nc.sync.dma_start(nat[:], win_d[ei * P:(ei + 1) * P, :])
                    transpose_pack4(
                        nc, W_inT, 0, ei * P,
                        lambda di, nat=nat: nat[:, di * P:(di + 1) * P],
                        DT, ident, f32)
                for di in range(DT):
                    nat2 = sp.tile([P, TWO_D], f32, tag="snat2")
                    nc.sync.dma_start(nat2[:], wout_d[di * P:(di + 1) * P, :])
                    natb = sp.tile([P, TWO_D], bf16, tag="snatb")
                    nc.scalar.activation(natb[:], nat2[:], Act.Copy)
                    transpose_pack4(
                        nc, W_outT, 0, di * P,
                        lambda ci, natb=natb: natb[:, ci * P:(ci + 1) * P],
                        CT, ident_bf, bf16)

            with (
                tc.tile_pool(name="ctx", bufs=1) as ctxp,
                tc.tile_pool(name="acts", bufs=1) as actsp,
                tc.tile_pool(name="rot", bufs=4) as natp,
                tc.tile_pool(name="sm", bufs=2) as smp,
            ):
                ctx_tiles = {}

                def emit_ctx_stage(b):
                    # context: transpose to ctxT (fp32r) + cast to bf16
                    ctxT = ctxp.tile([P, DT, K], f32r, tag="ctxT")     # [e,., k]
                    ctx_bf = ctxp.tile([P, KT, D], bf16, tag="ctxbf")  # [k,., d']
                    for ki in range(KT):
                        nat = natp.tile([P, D], f32, tag="nat")
                        nc.sync.dma_start(nat[:], c_d[b, ki * P:(ki + 1) * P, :])
                        nc.scalar.activation(ctx_bf[:, ki, :], nat[:], Act.Copy)
                        transpose_pack4(
                            nc, ctxT, 0, ki * P,
                            lambda ei, nat=nat: nat[:, ei * P:(ei + 1) * P],
                            DT, ident, f32)
                    ctx_tiles[b] = (ctxT, ctx_bf)

                def emit_query_loads(b, h):
                    l0 = h * LH
                    nats = []
                    for lj in range(LJ):
                        nat = natp.tile([P, D], f32, tag="nat")
                        nc.sync.dma_start(
                            nat[:], q_d[b, l0 + lj * P: l0 + (lj + 1) * P, :])
                        nats.append(nat)
                    return nats

                def emit_query_transposes(nats):
                    qT = actsp.tile([P, DT, LH], f32r, tag="qT")
                    for lj, nat in enumerate(nats):
                        transpose_pack4(
                            nc, qT, 0, lj * P,
                            lambda di, nat=nat: nat[:, di * P:(di + 1) * P],
                            DT, ident, f32)
                    return qT

                emit_ctx_stage(0)
                qT_next = emit_query_transposes(emit_query_loads(0, 0))

                for b in range(nb):
                    if b > 0:
                        # The bf16 cast must wait for the previous batch's
                        # step-4 reads of ctx_bf (bufs=1); emitting the whole
                        # stage here puts that wait harmlessly behind G(b-1)
                        # instead of blocking the ACT queue mid-batch.
                        emit_ctx_stage(b)
                    ctxT, ctx_bf = ctx_tiles.pop(b)
                    for h in range(NHALF):
                        l0 = h * LH
                        qT = qT_next

                        # ---- step 1: qTr[e, l] = W_inT.T @ qT (fp32r) ----
                        qTr = actsp.tile([P, DT, LH], f32r, tag="qTr")
                        qTr_bf = actsp.tile([P, DT, LH], bf16, tag="qTrbf")
                        for ei in range(DT):
                            psq = ps_mm.tile([P, LH], f32, tag="mm")
                            for di in range(DT):
                                nc.tensor.matmul(
                                    psq[:],
                                    W_inT[:, di, ei * P:(ei + 1) * P],
                                    qT[:, di, :],
                                    start=(di == 0), stop=(di == DT - 1),
                                )
                            nc.vector.tensor_copy(qTr[:, ei, :], psq[:])
                            nc.scalar.activation(qTr_bf[:, ei, :], psq[:], Act.Copy)

                        # Prefetch the next query tiles now so their DMAs sit
                        # ahead of this stage's mask loads in the queue.
                        if h + 1 < NHALF:
                            next_nats = emit_query_loads(b, h + 1)
                        elif b + 1 < nb:
                            next_nats = emit_query_loads(b + 1, 0)
                        else:
                            next_nats = None

                        # ---- step 2 + masked softmax; w transposes lag one
                        # l-tile so the softmax chain hides under the next
                        # tile's matmuls ----
                        wT = actsp.tile([P, KT, LH], bf16, tag="wT")
                        w_tiles = [None] * LJ

                        def emit_w_transpose(lj):
                            w_bf = w_tiles[lj]
                            for g in range(KT // 4):
                                tpb = ps_mm.tile([P, 4 * P], bf16, tag="mm")
                                for i in range(4):
                                    ki = g * 4 + i
                                    nc.tensor.transpose(
                                        tpb[:, i * P:(i + 1) * P],
                                        w_bf[:, ki * P:(ki + 1) * P], ident_bf[:])
                                grouped_copy(
                                    nc,
                                    wT[:, g * 4:(g + 1) * 4, lj * P:(lj + 1) * P],
                                    tpb[:])

                        for lj in range(LJ):
                            mi = smp.tile([P, K], i32, tag="mask")
                            nc.sync.dma_start(
                                mi[:], m_d[b, l0 + lj * P: l0 + (lj + 1) * P, :])
                            pss = ps_big.tile([P, K], f32, tag="scores")
                            for ei in range(DT):
                                for kh in range(KH):
                                    nc.tensor.matmul(
                                        pss[:, kh * 512:(kh + 1) * 512],
                                        qTr[:, ei, lj * P:(lj + 1) * P],
                                        ctxT[:, ei, kh * 512:(kh + 1) * 512],
                                        start=(ei == 0), stop=(ei == DT - 1),
                                    )
                            st = smp.tile([P, 4], f32, tag="stats")
                            # u = (s + LARGE) * m, in place in PSUM
                            nc.vector.scalar_tensor_tensor(
                                pss[:], pss[:], LARGE, mi[:],
                                op0=Alu.add, op1=Alu.mult)
                            nc.vector.tensor_reduce(
                                st[:, 0:1], pss[:], axis=mybir.AxisListType.X,
                                op=Alu.max, negate=True)
                            e_sb = smp.tile([P, K], bf16, tag="e")
                            nc.scalar.activation(
                                e_sb[:], pss[:], Act.Exp,
                                bias=st[:, 0:1], accum_out=st[:, 1:2])
                            nc.vector.reciprocal(st[:, 2:3], st[:, 1:2])
                            w_bf = smp.tile([P, K], bf16, tag="w")
                            nc.vector.tensor_scalar_mul(w_bf[:], e_sb[:], st[:, 2:3])
                            w_tiles[lj] = w_bf
                            if lj >= 1:
                                emit_w_transpose(lj - 1)

                        # Pre-emit the next query transposes to fill the last
                        # softmax tail before the final w transposes.
                        if next_nats is not None:
                            qT_next = emit_query_transposes(next_nats)
                        emit_w_transpose(LJ - 1)

                        # ---- step 4: mixT[d', l] = ctx_bf.T @ wT (bf16) ----
                        mixT = actsp.tile([P, DT, LH], bf16, tag="mixT")
                        for di in range(DT):
                            psm = ps_mm.tile([P, LH], f32, tag="mm")
                            for ki in range(KT):
                                nc.tensor.matmul(
                                    psm[:],
                                    ctx_bf[:, ki, di * P:(di + 1) * P],
                                    wT[:, ki, :],
                                    start=(ki == 0), stop=(ki == KT - 1),
                                )
                            nc.scalar.activation(mixT[:, di, :], psm[:], Act.Copy)

                        # ---- step 5: out[l, d] = tanh(combinedT.T @ W_outT) --
                        for lj in range(LJ):
                            for dh in range(D // 512):
                                pso = ps_out.tile([P, 512], f32, tag="out")
                                for ci in range(CT):
                                    if ci < DT:
                                        lhs = mixT[:, ci, lj * P:(lj + 1) * P]
                                    else:
                                        lhs = qTr_bf[:, ci - DT, lj * P:(lj + 1) * P]
                                    nc.tensor.matmul(
                                        pso[:], lhs,
                                        W_outT[:, ci, dh * 512:(dh + 1) * 512],
                                        start=(ci == 0), stop=(ci == CT - 1),
                                    )
                                o_sb = smp.tile([P, 512], f32, tag="osb")
                                nc.scalar.activation(o_sb[:], pso[:], Act.Tanh)
                                nc.sync.dma_start(
                                    out_d[b, l0 + lj * P: l0 + (lj + 1) * P,
                                          dh * 512:(dh + 1) * 512],
                                    o_sb[:])

    nc.compile()
    return nc


def _get_program(nb, L):
    key = (nb, L)
    if key not in _prog_cache:
        _prog_cache[key] = build_program(nb, L)
    return _prog_cache[key]


def kernel(query, context, mask, W_in, W_out):
    from concourse.bass_utils import run_bass_kernel_spmd

    query = np.ascontiguousarray(query, dtype=np.float32)
    context = np.ascontiguousarray(context, dtype=np.float32)
    W_in = np.ascontiguousarray(W_in, dtype=np.float32)
    W_out = np.ascontiguousarray(W_out, dtype=np.float32)
    B, L, _ = query.shape
    mask3 = np.ascontiguousarray(mask.reshape(B, L, -1), dtype=np.int32)

    nb = B // N_CORES
    nc = _get_program(nb, L)
    in_maps = []
    for c in range(N_CORES):
        b0 = c * nb
        in_maps.append({
            "query": query[b0:b0 + nb],
            "context": context[b0:b0 + nb],
            "mask": mask3[b0:b0 + nb],
            "W_in": W_in,
            "W_out": W_out,
        })
    res = run_bass_kernel_spmd(nc, in_maps, core_ids=list(range(N_CORES)))
    global last_results
    last_results = res
    out = np.concatenate([r["out"] for r in res.results], axis=0)
    return out
